# revision 8
# baseline (speedup 1.0000x reference)
"""Llama-3.2 attention block (T=2048, H=2048, 32 q heads / 8 kv heads, d=64)
as a Bass/Tile kernel on 8 Trainium2 NeuronCores.

Sharding: tensor-parallel over heads. Core c owns q heads 4c..4c+3 and kv
head c (the GQA group). Each core projects its QKV shard over the full
sequence, applies RoPE, runs causal attention for its 4 heads, then the
attention outputs are exchanged with an AllToAll so core c ends up with the
full head dimension for sequence chunk c. Each core then computes the o_proj
for its 256 sequence rows against the full (transposed) w_o and writes a
[256, 2048] fp32 slice of the output; the host concatenates the slices.

Layouts on device (bf16 matmul inputs, fp32 accumulation):
  - hidden and weights are pre-transposed on host so the contraction dim
    (hidden) lands on SBUF partitions.
  - QKV is produced transposed: q/k/v as [feat, seq] tiles. RoPE is applied
    in this layout: out = x * cos + (P @ x) * sin, where P is the
    rotate-half permutation done on the tensor engine.
  - q and k live in zero-padded [128, T] tiles (real head dim 64 on
    partitions 0:64, zeros on 64:128) so score matmuls contract over K=128;
    K=64 matmuls measure ~2.3x slower per moving column on TRN2.
  - scores are computed transposed (scoresT[k, q]) so softmax's exp runs on
    the scalar engine and P@V consumes probsT directly as the moving
    operand; the softmax denominator rides along as a ones-column appended
    to V. No max-subtraction is needed: |scores| <= ~20 for this problem,
    safely inside fp32 exp range. 1/denom is broadcast across partitions
    with a K=1 ones matmul on the tensor engine.
"""

import os
import sys
import types

import numpy as np
import ml_dtypes

T = 2048
HID = 2048
NH = 32
NKV = 8
D = 64
NCORES = 8
HPC = NH // NCORES        # q heads per core = 4
FPC = HPC * D             # attention feats per core = 256
SPC = T // NCORES         # seq chunk per core after AllToAll = 256
QKV_F = FPC + 2 * D       # per-core qkv proj feats = 384
ROPE_THETA = 500000.0
SCALE = float(D) ** -0.5

_CACHE = {}


def _ensure_trace_hooks():
    """Register the NTFF profiling hook that the stub antenv package lacks."""
    if "antenv.axon_hooks" in sys.modules:
        return
    try:
        import antenv
    except ImportError:
        return
    hooks = types.ModuleType("antenv.axon_hooks")
    holder = [None]
    hooks.set_axon_ntff_profile_hook = lambda h: holder.__setitem__(0, h)
    hooks.get_axon_ntff_profile_hook = lambda: holder[0]
    antenv.axon_hooks = hooks
    sys.modules["antenv.axon_hooks"] = hooks
    try:
        from trn_agent_boot.trn_boot import _ntff_profile_via_ctypes

        hook = _ntff_profile_via_ctypes("/opt/axon/libaxon_pjrt.so")
        if hook is not None:
            hooks.set_axon_ntff_profile_hook(hook)
    except Exception:
        pass


def _build():
    from contextlib import ExitStack

    from concourse import bacc
    import concourse.mybir as mybir
    import concourse.tile as tile
    from concourse.bass import ts
    from concourse.tile import add_dep_helper

    f32 = mybir.dt.float32
    bf16 = mybir.dt.bfloat16
    AF = mybir.ActivationFunctionType
    OP = mybir.AluOpType

    KO = HID // 128           # 16 contraction chunks
    NQ = T // 512             # 4 seq chunks of 512
    NB = T // 128             # 16 k blocks of 128

    nc = bacc.Bacc("TRN2", target_bir_lowering=False, debug=False, num_devices=NCORES)

    hT = nc.dram_tensor("hT", [HID, T], bf16, kind="ExternalInput")
    wT = nc.dram_tensor("wT", [HID, QKV_F], bf16, kind="ExternalInput")
    cosf = nc.dram_tensor("cosf", [128, T], f32, kind="ExternalInput")
    sinf = nc.dram_tensor("sinf", [128, T], f32, kind="ExternalInput")
    perm = nc.dram_tensor("perm", [128, 128], bf16, kind="ExternalInput")
    ident = nc.dram_tensor("ident", [128, 128], bf16, kind="ExternalInput")
    tri = nc.dram_tensor("tri", [128, 128], bf16, kind="ExternalInput")
    ones = nc.dram_tensor("ones", [128, 128], bf16, kind="ExternalInput")
    woT = nc.dram_tensor("woT", [HID, HID], bf16, kind="ExternalInput")
    out = nc.dram_tensor("out", [SPC, HID], f32, kind="ExternalOutput")
    a2a_in = [
        nc.dram_tensor(f"a2a_in{g}", [NCORES, FPC // 2, SPC], bf16) for g in range(2)
    ]
    a2a_out = [
        nc.dram_tensor(f"a2a_out{g}", [NCORES, FPC // 2, SPC], bf16) for g in range(2)
    ]

    with tile.TileContext(nc) as tc, ExitStack() as ctx:
        consts = ctx.enter_context(tc.tile_pool(name="consts", bufs=1))
        persist = ctx.enter_context(tc.tile_pool(name="persist", bufs=1))

        # first QKV matmuls are gated on wt + the first hT chunk; load those
        # first, in k order, so compute starts as early as possible
        wt_t = consts.tile([128, KO, QKV_F], bf16, tag="wt")
        ht0_t = consts.tile([128, KO, 512], bf16, tag="ht0")
        hT_re = hT.ap().rearrange("(ko p) s -> p ko s", p=128)
        for k4 in range(4):
            for k in range(4 * k4, 4 * k4 + 4):
                nc.sync.dma_start(wt_t[:, k, :], wT.ap()[ts(k, 128), :])
            nc.sync.dma_start(ht0_t[:, ts(k4, 4), :], hT_re[:, ts(k4, 4), 0:512])
        cos_t = consts.tile([128, T], f32, tag="cos")
        nc.sync.dma_start(cos_t, cosf.ap())
        sin_t = consts.tile([128, T], f32, tag="sin")
        nc.sync.dma_start(sin_t, sinf.ap())
        perm_t = consts.tile([128, 128], bf16, tag="perm")
        nc.sync.dma_start(perm_t, perm.ap())
        ident_t = consts.tile([128, 128], bf16, tag="ident")
        nc.sync.dma_start(ident_t, ident.ap())
        tri_t = consts.tile([128, 128], bf16, tag="tri")
        nc.sync.dma_start(tri_t, tri.ap())
        ones_t = consts.tile([128, 128], bf16, tag="ones")
        nc.sync.dma_start(ones_t, ones.ap())

        # Persistent activation tiles (live across phases A/B).
        q_t = [persist.tile([128, T], bf16, tag=f"q{p}", name=f"q{p}") for p in range(2)]
        qh_t = [persist.tile([128, T], bf16, tag=f"qh{h}", name=f"qh{h}") for h in range(HPC)]
        k_t = persist.tile([128, T], bf16, tag="kt")
        vlo_t = persist.tile([64, T], bf16, tag="vlo")
        vaug_t = persist.tile([128, KO, D + 1], bf16, tag="vaug")

        # zero the K-padding rows once
        for h in range(HPC):
            nc.vector.memset(qh_t[h][64:128, :], 0.0)
        nc.vector.memset(k_t[64:128, :], 0.0)

        # ---- Phase A: QKV projection + RoPE (outputs transposed [feat, seq]) ----
        with nc.named_scope("qkv"):
            with (
                tc.tile_pool(name="htp", bufs=2) as ht_pool,
                tc.tile_pool(name="atmp", bufs=3) as atmp,
                tc.tile_pool(name="psA", bufs=2, space="PSUM") as psA,
                tc.tile_pool(name="psAsh", bufs=2, space="PSUM") as psAsh,
            ):
                for n in range(NQ):
                    if n == 0:
                        ht_t = ht0_t
                    else:
                        ht_t = ht_pool.tile([128, KO, 512], bf16, tag="ht")
                        nc.sync.dma_start(
                            ht_t,
                            hT.ap()[:, ts(n, 512)].rearrange("(ko p) s -> p ko s", p=128),
                        )
                    for m in range(3):
                        pq = psA.tile([128, 512], f32, tag="pq")
                        for k in range(KO):
                            nc.tensor.matmul(
                                pq,
                                wt_t[:, k, ts(m, 128)],
                                ht_t[:, k, :],
                                start=(k == 0),
                                stop=(k == KO - 1),
                            )
                        xb = atmp.tile([128, 512], bf16, tag="xb")
                        nc.vector.tensor_copy(xb, pq)
                        if m < 2:
                            # two q heads: rotate-half via PE perm, combine on DVE
                            psh = psAsh.tile([128, 512], f32, tag="psh")
                            nc.tensor.matmul(psh, perm_t, xb, start=True, stop=True)
                            t1 = atmp.tile([128, 512], f32, tag="t1")
                            nc.vector.tensor_tensor(t1, xb, cos_t[:, ts(n, 512)], OP.mult)
                            t2 = atmp.tile([128, 512], f32, tag="t2")
                            nc.vector.tensor_tensor(t2, psh, sin_t[:, ts(n, 512)], OP.mult)
                            nc.vector.tensor_tensor(q_t[m][:, ts(n, 512)], t1, t2, OP.add)
                            for hh in range(2):
                                nc.sync.dma_start(
                                    qh_t[2 * m + hh][0:64, ts(n, 512)],
                                    q_t[m][hh * 64:hh * 64 + 64, ts(n, 512)],
                                )
                        else:
                            # k head on partitions 0:64 (rope), v head on 64:128 (plain)
                            psh = psAsh.tile([128, 512], f32, tag="psh")
                            nc.tensor.matmul(
                                psh[0:64, :], perm_t[0:64, 0:64], xb[0:64, :],
                                start=True, stop=True,
                            )
                            t1 = atmp.tile([128, 512], f32, tag="t1")
                            nc.vector.tensor_tensor(
                                t1[0:64, :], xb[0:64, :], cos_t[0:64, ts(n, 512)], OP.mult
                            )
                            t2 = atmp.tile([128, 512], f32, tag="t2")
                            nc.vector.tensor_tensor(
                                t2[0:64, :], psh[0:64, :], sin_t[0:64, ts(n, 512)], OP.mult
                            )
                            nc.vector.tensor_tensor(
                                k_t[0:64, ts(n, 512)], t1[0:64, :], t2[0:64, :], OP.add
                            )
                            # v slice to partitions 0:64 via DMA (partition remap)
                            nc.sync.dma_start(vlo_t[:, ts(n, 512)], xb[64:128, :])


            # v natural layout [seq, d] + ones column for the denominator
            with tc.tile_pool(name="psV", bufs=2, space="PSUM") as psV:
                nc.vector.memset(vaug_t[:, :, D:D + 1], 1.0)
                for j in range(KO):
                    pv = psV.tile([128, D], bf16, tag="pv")
                    nc.tensor.transpose(pv, vlo_t[:, ts(j, 128)], ident_t[0:64, 0:64])
                    nc.vector.tensor_copy(vaug_t[:, j, 0:D], pv)

        # w_o load kicked off here so it overlaps attention (contiguous per-k chunks)
        wo_t = consts.tile([128, KO, HID], bf16, tag="wo")
        for k in range(KO):
            nc.sync.dma_start(wo_t[:, k, :], woT.ap()[ts(k, 128), :])

        # ---- Phase B: causal attention, 4 heads, scoresT layout ----
        a2a_dmas = []
        with nc.named_scope("attn"):
            with (
                tc.tile_pool(name="probs", bufs=2) as probs_pool,
                tc.tile_pool(name="btmp", bufs=4) as btmp,
                tc.tile_pool(name="psS", bufs=2, space="PSUM") as psS,
                tc.tile_pool(name="psO", bufs=2, space="PSUM") as psO,
                tc.tile_pool(name="psB", bufs=2, space="PSUM") as psB,
            ):
                for h in range(HPC):
                    for i in range(NQ):
                        nj = 4 * i + 4
                        pr = probs_pool.tile([128, NB, 512], bf16, tag="pr")
                        po = psO.tile([D + 1, 512], f32, tag="po")
                        j = 0
                        while j < nj:
                            r = j - 4 * i
                            if r < -1:
                                # two full-width blocks share one psum tile and
                                # one exp call
                                pss = psS.tile([128, 2, 512], f32, tag="pss")
                                for u in range(2):
                                    nc.tensor.matmul(
                                        pss[:, u, :],
                                        k_t[:, ts(j + u, 128)],
                                        qh_t[h][:, ts(i, 512)],
                                        start=True, stop=True,
                                    )
                                nc.scalar.activation(
                                    pr[:, j:j + 2, :], pss, AF.Exp, scale=SCALE
                                )
                                j += 2
                                continue
                            off = max(0, r) * 128
                            pss = psS.tile([128, 2, 512], f32, tag="pss")
                            nc.tensor.matmul(
                                pss[:, 0, off:512],
                                k_t[:, ts(j, 128)],
                                qh_t[h][:, i * 512 + off:(i + 1) * 512],
                                start=True, stop=True,
                            )
                            nc.scalar.activation(
                                pr[:, j, off:512], pss[:, 0, off:512], AF.Exp, scale=SCALE
                            )
                            if r >= 0:  # block overlapping the causal diagonal
                                nc.vector.tensor_tensor(
                                    pr[:, j, off:off + 128], pr[:, j, off:off + 128],
                                    tri_t, OP.mult,
                                )
                            j += 1
                        for j in range(nj):
                            off = max(0, j - 4 * i) * 128
                            nc.tensor.matmul(
                                po[:, off:512], vaug_t[:, j, :], pr[:, j, off:512],
                                start=(j == 0), stop=(j == nj - 1),
                            )
                        # normalize: oT[f, q] = po[f, q] / den[q]; den row broadcast
                        # across partitions via a K=1 ones matmul, then 1/x on DVE
                        dbc = btmp.tile([D + 1, 512], bf16, tag="dbc")
                        nc.vector.tensor_copy(dbc[D:D + 1, :], po[D:D + 1, :])
                        pb = psB.tile([D, 512], f32, tag="pb")
                        nc.tensor.matmul(
                            pb, ones_t[D:D + 1, 0:D], dbc[D:D + 1, :],
                            start=True, stop=True,
                        )
                        rbs = btmp.tile([D, 512], f32, tag="rbs")
                        nc.vector.reciprocal_approx_fast(out=rbs, in_=pb)
                        oth = btmp.tile([D, 512], bf16, tag="oth")
                        nc.vector.tensor_tensor(oth, po[0:D, :], rbs, OP.mult)
                        for half in range(2):
                            dd = nc.sync.dma_start(
                                a2a_in[h // 2].ap()[2 * i + half, ts(h % 2, D), :],
                                oth[:, ts(half, 256)],
                            )
                            a2a_dmas.append((h // 2, dd))

        # ---- Phase C: AllToAlls over the head dim -> seq-sharded full-head oT.
        # Two collectives, one per head pair, so the first overlaps the second
        # pair's attention and o_proj's even-k half overlaps the second.
        ccs = []
        for g in range(2):
            cc = nc.gpsimd.collective_compute(
                "AllToAll",
                OP.bypass,
                replica_groups=[list(range(NCORES))],
                ins=[a2a_in[g].ap()],
                outs=[a2a_out[g].ap()],
            )
            for gg, dd in a2a_dmas:
                if gg == g:
                    add_dep_helper(cc.ins, dd.ins, sync=True, reason="cc waits a2a stage-in")
            ccs.append(cc)

        # ---- Phase D: o_proj for this core's 256 seq rows ----
        with nc.named_scope("oproj"):
            with (
                tc.tile_pool(name="lo", bufs=1) as lo_pool,
                tc.tile_pool(name="dtmp", bufs=3) as dtmp,
                tc.tile_pool(name="psD", bufs=2, space="PSUM") as psD,
            ):
                lo_t = lo_pool.tile([128, KO, SPC], bf16, tag="lo")
                a2a_flat = [a2a_out[g].ap().rearrange("a f s -> (a f) s") for g in range(2)]
                for k in range(KO):
                    g, src = k % 2, k // 2
                    dl = nc.sync.dma_start(lo_t[:, k, :], a2a_flat[g][ts(src, 128), :])
                    add_dep_helper(dl.ins, ccs[g].ins, sync=True, reason="o_proj waits AllToAll")
                # even k chunks depend only on the first AllToAll; run them
                # while the second collective is still in flight
                korder = [k for k in range(KO) if k % 2 == 0] + [
                    k for k in range(KO) if k % 2 == 1
                ]
                for m in range(SPC // 128):
                    for e4 in range(HID // 512):
                        pso = psD.tile([128, 512], f32, tag="pso")
                        for ki, k in enumerate(korder):
                            nc.tensor.matmul(
                                pso,
                                lo_t[:, k, ts(m, 128)],
                                wo_t[:, k, ts(e4, 512)],
                                start=(ki == 0),
                                stop=(ki == KO - 1),
                            )
                        ob = dtmp.tile([128, 512], f32, tag="ob")
                        nc.vector.tensor_copy(ob, pso)
                        nc.sync.dma_start(out.ap()[ts(m, 128), ts(e4, 512)], ob)

    nc.compile()
    return nc


def _get_nc():
    if "nc" not in _CACHE:
        _CACHE["nc"] = _build()
    return _CACHE["nc"]


def _host_prep(hidden_states, positions, w_qkv, w_o):
    bf16 = ml_dtypes.bfloat16
    hTb = np.ascontiguousarray(hidden_states.astype(np.float32).T).astype(bf16)
    woTb = np.ascontiguousarray(w_o.astype(np.float32).T).astype(bf16)

    inv = 1.0 / (ROPE_THETA ** (np.arange(0, D, 2, dtype=np.float32) / D))  # [32]
    ang = positions.astype(np.float32)[:, None] * inv[None, :]              # [T, 32]
    cos = np.cos(ang).T  # [32, T]
    sin = np.sin(ang).T
    p = np.arange(128)
    fr = (p % D) % (D // 2)
    sgn = np.where((p % D) < (D // 2), -1.0, 1.0).astype(np.float32)
    cosf = np.ascontiguousarray(cos[fr])                     # [128, T]
    sinf = np.ascontiguousarray(sin[fr] * sgn[:, None])      # [128, T]

    partner = np.where((p % D) < (D // 2), p + D // 2, p - D // 2)
    perm = np.zeros((128, 128), dtype=np.float32)
    perm[p, partner] = 1.0
    ident = np.eye(128, dtype=np.float32)
    tri = (np.arange(128)[None, :] >= np.arange(128)[:, None]).astype(np.float32)
    ones_m = np.ones((128, 128), dtype=np.float32)

    q_size = NH * D
    kv_size = NKV * D
    in_maps = []
    for c in range(NCORES):
        wq = w_qkv[c * FPC:(c + 1) * FPC]
        wk = w_qkv[q_size + c * D:q_size + (c + 1) * D]
        wv = w_qkv[q_size + kv_size + c * D:q_size + kv_size + (c + 1) * D]
        wTc = np.ascontiguousarray(
            np.concatenate([wq, wk, wv], axis=0).astype(np.float32).T
        ).astype(bf16)
        in_maps.append(
            {
                "hT": hTb,
                "wT": wTc,
                "cosf": cosf,
                "sinf": sinf,
                "perm": perm.astype(bf16),
                "ident": ident.astype(bf16),
                "tri": tri.astype(bf16),
                "ones": ones_m.astype(bf16),
                "woT": woTb,
            }
        )
    return in_maps


def run(inputs, trace=False):
    """Run on 8 NeuronCores; returns (full_output, BassKernelResults)."""
    if trace:
        _ensure_trace_hooks()
    from concourse import bass_utils

    if trace:
        bass_utils.upload_artifacts = lambda tmpdir: tmpdir
    nc = _get_nc()
    in_maps = _host_prep(
        np.asarray(inputs["hidden_states"]),
        np.asarray(inputs["positions"]),
        np.asarray(inputs["w_qkv"]),
        np.asarray(inputs["w_o"]),
    )
    res = bass_utils.run_bass_kernel_spmd(
        nc, in_maps, core_ids=list(range(NCORES)), trace=trace
    )
    full = np.concatenate(
        [res.results[c]["out"] for c in range(NCORES)], axis=0
    ).astype(np.float32)
    return full, res


def kernel(**inputs) -> np.ndarray:
    trace = bool(os.environ.get("KERNEL_TRACE"))
    full, _ = run(inputs, trace=trace)
    return full


# revision 9
# speedup vs baseline: 1.0932x; 1.0932x over previous
"""Llama-3.2 attention block (T=2048, H=2048, 32 q heads / 8 kv heads, d=64)
as a Bass/Tile kernel on 8 Trainium2 NeuronCores.

Sharding: tensor-parallel over heads. Core c owns q heads 4c..4c+3 and kv
head c (the GQA group). Each core projects its QKV shard over the full
sequence, applies RoPE, runs causal attention for its 4 heads, then the
attention outputs are exchanged with an AllToAll so core c ends up with the
full head dimension for sequence chunk c. Each core then computes the o_proj
for its 256 sequence rows against the full (transposed) w_o and writes a
[256, 2048] fp32 slice of the output; the host concatenates the slices.

Layouts on device (bf16 matmul inputs, fp32 accumulation):
  - hidden and weights are pre-transposed on host so the contraction dim
    (hidden) lands on SBUF partitions.
  - QKV is produced transposed: q/k/v as [feat, seq] tiles. RoPE is applied
    in this layout: out = x * cos + (P @ x) * sin, where P is the
    rotate-half permutation done on the tensor engine.
  - q and k live in zero-padded [128, T] tiles (real head dim 64 on
    partitions 0:64, zeros on 64:128) so score matmuls contract over K=128;
    K=64 matmuls measure ~2.3x slower per moving column on TRN2.
  - scores are computed transposed (scoresT[k, q]) so softmax's exp runs on
    the scalar engine and P@V consumes probsT directly as the moving
    operand; the softmax denominator rides along as a ones-column appended
    to V. No max-subtraction is needed: |scores| <= ~20 for this problem,
    safely inside fp32 exp range. 1/denom is broadcast across partitions
    with a K=1 ones matmul on the tensor engine.
"""

import os
import sys
import types

import numpy as np
import ml_dtypes

T = 2048
HID = 2048
NH = 32
NKV = 8
D = 64
NCORES = 8
HPC = NH // NCORES        # q heads per core = 4
FPC = HPC * D             # attention feats per core = 256
SPC = T // NCORES         # seq chunk per core after AllToAll = 256
QKV_F = FPC + 2 * D       # per-core qkv proj feats = 384
ROPE_THETA = 500000.0
SCALE = float(D) ** -0.5

_CACHE = {}


def _ensure_trace_hooks():
    """Register the NTFF profiling hook that the stub antenv package lacks."""
    if "antenv.axon_hooks" in sys.modules:
        return
    try:
        import antenv
    except ImportError:
        return
    hooks = types.ModuleType("antenv.axon_hooks")
    holder = [None]
    hooks.set_axon_ntff_profile_hook = lambda h: holder.__setitem__(0, h)
    hooks.get_axon_ntff_profile_hook = lambda: holder[0]
    antenv.axon_hooks = hooks
    sys.modules["antenv.axon_hooks"] = hooks
    try:
        from trn_agent_boot.trn_boot import _ntff_profile_via_ctypes

        hook = _ntff_profile_via_ctypes("/opt/axon/libaxon_pjrt.so")
        if hook is not None:
            hooks.set_axon_ntff_profile_hook(hook)
    except Exception:
        pass


def _build():
    from contextlib import ExitStack

    from concourse import bacc
    import concourse.mybir as mybir
    import concourse.tile as tile
    from concourse.bass import ts
    from concourse.tile import add_dep_helper

    f32 = mybir.dt.float32
    bf16 = mybir.dt.bfloat16
    AF = mybir.ActivationFunctionType
    OP = mybir.AluOpType

    KO = HID // 128           # 16 contraction chunks
    NQ = T // 512             # 4 seq chunks of 512
    NB = T // 128             # 16 k blocks of 128

    nc = bacc.Bacc("TRN2", target_bir_lowering=False, debug=False, num_devices=NCORES)

    hT = nc.dram_tensor("hT", [HID, T], bf16, kind="ExternalInput")
    wT = nc.dram_tensor("wT", [HID, QKV_F], bf16, kind="ExternalInput")
    cosf = nc.dram_tensor("cosf", [128, T], f32, kind="ExternalInput")
    sinf = nc.dram_tensor("sinf", [128, T], f32, kind="ExternalInput")
    perm = nc.dram_tensor("perm", [128, 128], bf16, kind="ExternalInput")
    ident = nc.dram_tensor("ident", [128, 128], bf16, kind="ExternalInput")
    tri = nc.dram_tensor("tri", [128, 128], bf16, kind="ExternalInput")
    ones = nc.dram_tensor("ones", [128, 128], bf16, kind="ExternalInput")
    woT = nc.dram_tensor("woT", [HID, HID], bf16, kind="ExternalInput")
    out = nc.dram_tensor("out", [SPC, HID], f32, kind="ExternalOutput")
    a2a_in = [
        nc.dram_tensor(f"a2a_in{g}", [NCORES, FPC // 2, SPC], bf16) for g in range(2)
    ]
    a2a_out = [
        nc.dram_tensor(f"a2a_out{g}", [NCORES, FPC // 2, SPC], bf16) for g in range(2)
    ]

    with tile.TileContext(nc) as tc, ExitStack() as ctx:
        consts = ctx.enter_context(tc.tile_pool(name="consts", bufs=1))
        persist = ctx.enter_context(tc.tile_pool(name="persist", bufs=1))

        # first QKV matmuls are gated on wt + the first hT chunk; load those
        # first, in k order, so compute starts as early as possible
        wt_t = consts.tile([128, KO, QKV_F], bf16, tag="wt")
        ht0_t = consts.tile([128, KO, 512], bf16, tag="ht0")
        hT_re = hT.ap().rearrange("(ko p) s -> p ko s", p=128)
        for k4 in range(4):
            for k in range(4 * k4, 4 * k4 + 4):
                nc.sync.dma_start(wt_t[:, k, :], wT.ap()[ts(k, 128), :])
            nc.sync.dma_start(ht0_t[:, ts(k4, 4), :], hT_re[:, ts(k4, 4), 0:512])
        cos_t = consts.tile([128, T], f32, tag="cos")
        nc.sync.dma_start(cos_t, cosf.ap())
        sin_t = consts.tile([128, T], f32, tag="sin")
        nc.sync.dma_start(sin_t, sinf.ap())
        perm_t = consts.tile([128, 128], bf16, tag="perm")
        nc.sync.dma_start(perm_t, perm.ap())
        ident_t = consts.tile([128, 128], bf16, tag="ident")
        nc.sync.dma_start(ident_t, ident.ap())
        tri_t = consts.tile([128, 128], bf16, tag="tri")
        nc.sync.dma_start(tri_t, tri.ap())
        ones_t = consts.tile([128, 128], bf16, tag="ones")
        nc.sync.dma_start(ones_t, ones.ap())

        # Persistent activation tiles (live across phases A/B).
        q_t = [persist.tile([128, T], bf16, tag=f"q{p}", name=f"q{p}") for p in range(2)]
        qh_t = [persist.tile([128, T], bf16, tag=f"qh{h}", name=f"qh{h}") for h in range(HPC)]
        k_t = persist.tile([128, T], bf16, tag="kt")
        vlo_t = persist.tile([64, T], bf16, tag="vlo")
        vaug_t = persist.tile([128, KO, D + 1], bf16, tag="vaug")

        # zero the K-padding rows once
        for h in range(HPC):
            nc.vector.memset(qh_t[h][64:128, :], 0.0)
        nc.vector.memset(k_t[64:128, :], 0.0)

        # ---- Phase A: QKV projection + RoPE (outputs transposed [feat, seq]) ----
        with nc.named_scope("qkv"):
            with (
                tc.tile_pool(name="htp", bufs=2) as ht_pool,
                tc.tile_pool(name="atmp", bufs=3) as atmp,
                tc.tile_pool(name="psA", bufs=2, space="PSUM") as psA,
                tc.tile_pool(name="psAsh", bufs=2, space="PSUM") as psAsh,
            ):
                for n in range(NQ):
                    if n == 0:
                        ht_t = ht0_t
                    else:
                        ht_t = ht_pool.tile([128, KO, 512], bf16, tag="ht")
                        nc.sync.dma_start(
                            ht_t,
                            hT.ap()[:, ts(n, 512)].rearrange("(ko p) s -> p ko s", p=128),
                        )
                    for m in range(3):
                        pq = psA.tile([128, 512], f32, tag="pq")
                        for k in range(KO):
                            nc.tensor.matmul(
                                pq,
                                wt_t[:, k, ts(m, 128)],
                                ht_t[:, k, :],
                                start=(k == 0),
                                stop=(k == KO - 1),
                            )
                        xb = atmp.tile([128, 512], bf16, tag="xb")
                        nc.vector.tensor_copy(xb, pq)
                        if m < 2:
                            # two q heads: rotate-half via PE perm, combine on DVE
                            psh = psAsh.tile([128, 512], f32, tag="psh")
                            nc.tensor.matmul(psh, perm_t, xb, start=True, stop=True)
                            t1 = atmp.tile([128, 512], f32, tag="t1")
                            nc.vector.tensor_tensor(t1, xb, cos_t[:, ts(n, 512)], OP.mult)
                            t2 = atmp.tile([128, 512], f32, tag="t2")
                            nc.vector.tensor_tensor(t2, psh, sin_t[:, ts(n, 512)], OP.mult)
                            nc.vector.tensor_tensor(q_t[m][:, ts(n, 512)], t1, t2, OP.add)
                            for hh in range(2):
                                nc.sync.dma_start(
                                    qh_t[2 * m + hh][0:64, ts(n, 512)],
                                    q_t[m][hh * 64:hh * 64 + 64, ts(n, 512)],
                                )
                        else:
                            # k head on partitions 0:64 (rope), v head on 64:128 (plain)
                            psh = psAsh.tile([128, 512], f32, tag="psh")
                            nc.tensor.matmul(
                                psh[0:64, :], perm_t[0:64, 0:64], xb[0:64, :],
                                start=True, stop=True,
                            )
                            t1 = atmp.tile([128, 512], f32, tag="t1")
                            nc.vector.tensor_tensor(
                                t1[0:64, :], xb[0:64, :], cos_t[0:64, ts(n, 512)], OP.mult
                            )
                            t2 = atmp.tile([128, 512], f32, tag="t2")
                            nc.vector.tensor_tensor(
                                t2[0:64, :], psh[0:64, :], sin_t[0:64, ts(n, 512)], OP.mult
                            )
                            nc.vector.tensor_tensor(
                                k_t[0:64, ts(n, 512)], t1[0:64, :], t2[0:64, :], OP.add
                            )
                            # v slice to partitions 0:64 via DMA (partition remap)
                            nc.sync.dma_start(vlo_t[:, ts(n, 512)], xb[64:128, :])


            # v natural layout [seq, d] + ones column for the denominator
            with tc.tile_pool(name="psV", bufs=2, space="PSUM") as psV:
                nc.vector.memset(vaug_t[:, :, D:D + 1], 1.0)
                for j in range(KO):
                    pv = psV.tile([128, D], bf16, tag="pv")
                    nc.tensor.transpose(pv, vlo_t[:, ts(j, 128)], ident_t[0:64, 0:64])
                    nc.vector.tensor_copy(vaug_t[:, j, 0:D], pv)

        wo_t = consts.tile([128, KO, HID], bf16, tag="wo")

        # ---- Phase B: causal attention, 4 heads, scoresT layout ----
        a2a_dmas = []
        with nc.named_scope("attn"):
            with (
                tc.tile_pool(name="probs", bufs=2) as probs_pool,
                tc.tile_pool(name="btmp", bufs=4) as btmp,
                tc.tile_pool(name="psS", bufs=2, space="PSUM") as psS,
                tc.tile_pool(name="psO", bufs=2, space="PSUM") as psO,
                tc.tile_pool(name="psB", bufs=2, space="PSUM") as psB,
            ):
                for h in range(HPC):
                    if h == 1:
                        # queues are quiet now; stream in w_o for phase D
                        for k in range(KO):
                            nc.sync.dma_start(wo_t[:, k, :], woT.ap()[ts(k, 128), :])
                    for i in range(NQ):
                        nj = 4 * i + 4
                        pr = probs_pool.tile([128, NB, 512], bf16, tag="pr")
                        po = psO.tile([D + 1, 512], f32, tag="po")
                        j = 0
                        while j < nj:
                            r = j - 4 * i
                            if r < -1:
                                # two full-width blocks share one psum tile and
                                # one exp call
                                pss = psS.tile([128, 2, 512], f32, tag="pss")
                                for u in range(2):
                                    nc.tensor.matmul(
                                        pss[:, u, :],
                                        k_t[:, ts(j + u, 128)],
                                        qh_t[h][:, ts(i, 512)],
                                        start=True, stop=True,
                                    )
                                nc.scalar.activation(
                                    pr[:, j:j + 2, :], pss, AF.Exp, scale=SCALE
                                )
                                j += 2
                                continue
                            off = max(0, r) * 128
                            pss = psS.tile([128, 2, 512], f32, tag="pss")
                            nc.tensor.matmul(
                                pss[:, 0, off:512],
                                k_t[:, ts(j, 128)],
                                qh_t[h][:, i * 512 + off:(i + 1) * 512],
                                start=True, stop=True,
                            )
                            nc.scalar.activation(
                                pr[:, j, off:512], pss[:, 0, off:512], AF.Exp, scale=SCALE
                            )
                            if r >= 0:  # block overlapping the causal diagonal
                                nc.vector.tensor_tensor(
                                    pr[:, j, off:off + 128], pr[:, j, off:off + 128],
                                    tri_t, OP.mult,
                                )
                            j += 1
                        for j in range(nj):
                            off = max(0, j - 4 * i) * 128
                            nc.tensor.matmul(
                                po[:, off:512], vaug_t[:, j, :], pr[:, j, off:512],
                                start=(j == 0), stop=(j == nj - 1),
                            )
                        # normalize: oT[f, q] = po[f, q] / den[q]; den row broadcast
                        # across partitions via a K=1 ones matmul, then 1/x on DVE
                        dbc = btmp.tile([D + 1, 512], bf16, tag="dbc")
                        nc.vector.tensor_copy(dbc[D:D + 1, :], po[D:D + 1, :])
                        pb = psB.tile([D, 512], f32, tag="pb")
                        nc.tensor.matmul(
                            pb, ones_t[D:D + 1, 0:D], dbc[D:D + 1, :],
                            start=True, stop=True,
                        )
                        rbs = btmp.tile([D, 512], f32, tag="rbs")
                        nc.vector.reciprocal_approx_fast(out=rbs, in_=pb)
                        oth = btmp.tile([D, 512], bf16, tag="oth")
                        nc.vector.tensor_tensor(oth, po[0:D, :], rbs, OP.mult)
                        for half in range(2):
                            dd = nc.sync.dma_start(
                                a2a_in[h // 2].ap()[2 * i + half, ts(h % 2, D), :],
                                oth[:, ts(half, 256)],
                            )
                            a2a_dmas.append((h // 2, dd))

        # ---- Phase C: AllToAlls over the head dim -> seq-sharded full-head oT.
        # Two collectives, one per head pair, so the first overlaps the second
        # pair's attention and o_proj's even-k half overlaps the second.
        ccs = []
        for g in range(2):
            cc = nc.gpsimd.collective_compute(
                "AllToAll",
                OP.bypass,
                replica_groups=[list(range(NCORES))],
                ins=[a2a_in[g].ap()],
                outs=[a2a_out[g].ap()],
            )
            for gg, dd in a2a_dmas:
                if gg == g:
                    add_dep_helper(cc.ins, dd.ins, sync=True, reason="cc waits a2a stage-in")
            ccs.append(cc)

        # ---- Phase D: o_proj for this core's 256 seq rows ----
        with nc.named_scope("oproj"):
            with (
                tc.tile_pool(name="lo", bufs=1) as lo_pool,
                tc.tile_pool(name="dtmp", bufs=3) as dtmp,
                tc.tile_pool(name="psD", bufs=8, space="PSUM") as psD,
            ):
                lo_t = lo_pool.tile([128, KO, SPC], bf16, tag="lo")
                a2a_flat = [a2a_out[g].ap().rearrange("a f s -> (a f) s") for g in range(2)]
                for k in range(KO):
                    g, src = k % 2, k // 2
                    dl = nc.sync.dma_start(lo_t[:, k, :], a2a_flat[g][ts(src, 128), :])
                    add_dep_helper(dl.ins, ccs[g].ins, sync=True, reason="o_proj waits AllToAll")
                # even k chunks depend only on the first AllToAll; run them
                # while the second collective is still in flight
                korder = [k for k in range(KO) if k % 2 == 0] + [
                    k for k in range(KO) if k % 2 == 1
                ]
                for m in range(SPC // 128):
                    for e4 in range(HID // 512):
                        pso = psD.tile([128, 512], f32, tag="pso")
                        for ki, k in enumerate(korder):
                            nc.tensor.matmul(
                                pso,
                                lo_t[:, k, ts(m, 128)],
                                wo_t[:, k, ts(e4, 512)],
                                start=(ki == 0),
                                stop=(ki == KO - 1),
                            )
                        ob = dtmp.tile([128, 512], f32, tag="ob")
                        nc.vector.tensor_copy(ob, pso)
                        nc.sync.dma_start(out.ap()[ts(m, 128), ts(e4, 512)], ob)

    nc.compile()
    return nc


def _get_nc():
    if "nc" not in _CACHE:
        _CACHE["nc"] = _build()
    return _CACHE["nc"]


def _host_prep(hidden_states, positions, w_qkv, w_o):
    bf16 = ml_dtypes.bfloat16
    hTb = np.ascontiguousarray(hidden_states.astype(np.float32).T).astype(bf16)
    woTb = np.ascontiguousarray(w_o.astype(np.float32).T).astype(bf16)

    inv = 1.0 / (ROPE_THETA ** (np.arange(0, D, 2, dtype=np.float32) / D))  # [32]
    ang = positions.astype(np.float32)[:, None] * inv[None, :]              # [T, 32]
    cos = np.cos(ang).T  # [32, T]
    sin = np.sin(ang).T
    p = np.arange(128)
    fr = (p % D) % (D // 2)
    sgn = np.where((p % D) < (D // 2), -1.0, 1.0).astype(np.float32)
    cosf = np.ascontiguousarray(cos[fr])                     # [128, T]
    sinf = np.ascontiguousarray(sin[fr] * sgn[:, None])      # [128, T]

    partner = np.where((p % D) < (D // 2), p + D // 2, p - D // 2)
    perm = np.zeros((128, 128), dtype=np.float32)
    perm[p, partner] = 1.0
    ident = np.eye(128, dtype=np.float32)
    tri = (np.arange(128)[None, :] >= np.arange(128)[:, None]).astype(np.float32)
    ones_m = np.ones((128, 128), dtype=np.float32)

    q_size = NH * D
    kv_size = NKV * D
    in_maps = []
    for c in range(NCORES):
        wq = w_qkv[c * FPC:(c + 1) * FPC]
        wk = w_qkv[q_size + c * D:q_size + (c + 1) * D]
        wv = w_qkv[q_size + kv_size + c * D:q_size + kv_size + (c + 1) * D]
        wTc = np.ascontiguousarray(
            np.concatenate([wq, wk, wv], axis=0).astype(np.float32).T
        ).astype(bf16)
        in_maps.append(
            {
                "hT": hTb,
                "wT": wTc,
                "cosf": cosf,
                "sinf": sinf,
                "perm": perm.astype(bf16),
                "ident": ident.astype(bf16),
                "tri": tri.astype(bf16),
                "ones": ones_m.astype(bf16),
                "woT": woTb,
            }
        )
    return in_maps


def run(inputs, trace=False):
    """Run on 8 NeuronCores; returns (full_output, BassKernelResults)."""
    if trace:
        _ensure_trace_hooks()
    from concourse import bass_utils

    if trace:
        bass_utils.upload_artifacts = lambda tmpdir: tmpdir
    nc = _get_nc()
    in_maps = _host_prep(
        np.asarray(inputs["hidden_states"]),
        np.asarray(inputs["positions"]),
        np.asarray(inputs["w_qkv"]),
        np.asarray(inputs["w_o"]),
    )
    res = bass_utils.run_bass_kernel_spmd(
        nc, in_maps, core_ids=list(range(NCORES)), trace=trace
    )
    full = np.concatenate(
        [res.results[c]["out"] for c in range(NCORES)], axis=0
    ).astype(np.float32)
    return full, res


def kernel(**inputs) -> np.ndarray:
    trace = bool(os.environ.get("KERNEL_TRACE"))
    full, _ = run(inputs, trace=trace)
    return full
